# revision 6
# baseline (speedup 1.0000x reference)
"""Trainium2 Bass kernel for nn_EnokeeEncoder (segment_reduce).

Reference semantics:
    lhs = embed[input_ids]                      # only lhs[:, :32, :] is ever used
    m[b,j,x] = (pos[b,j,x] != -1) & (am[b,j] != 0)
    pooled = einsum('bml,bld->bmd', m, lhs[:, :32]) / 32
    x = LayerNorm(pooled) * gamma + beta
    out = (x @ w1) @ w2 + b2                    # [16, 64, 100000]

Device strategy (8 cores, SPMD, no collectives):
  - mention rows whose mask is all-zero (am==0 or empty prefix) produce the
    constant row (beta @ w1) @ w2 + b2 — those (~half) are filled on the
    host. Only the active mentions are computed on device, compacted into
    TP = 4*GC token columns (GC = per-batch-group column count, 128 here).
  - the block-diagonal pooling mask is built on the host and DMA'd (bf16),
    removing all device-side mask construction.
  - every core redundantly computes hT [R=128, TP] (cheap) with the
    LayerNorm folded algebraically:
        h = rs * y + (-rs*mu) * u + c
    with y = pooled @ w1g, w1g = gamma (.) w1, u = gamma @ w1, c = beta @ w1,
    and rs/mu per-token stats of pooled (partition reductions via
    ones-matmuls on the PE).
  - the output projection is tensor-parallel over the entity vocab:
    core c computes out[:, c*12500:(c+1)*12500] = hT.T @ w2[:, shard].
  - embeddings, w2, hT and the output are bf16 (tolerance is 2e-2; bf16
    contributes ~4e-3): output DMA bytes drop 4x vs the fp32 full-token
    version, and the kernel is output-DMA-bound.
"""

import sys

if '/opt/trn_rl_repo' not in sys.path:
    sys.path.insert(0, '/opt/trn_rl_repo')

import numpy as np
import ml_dtypes

import concourse.bass as bass
import concourse.mybir as mybir
import concourse.tile as tile
from concourse import bacc
from concourse.bass_utils import run_bass_kernel_spmd

# model dims (fixed by the problem)
B, S, M, L, D = 16, 512, 64, 32, 1024
V, R, E = 32000, 128, 100000
LN_EPS = 1e-5

N_CORES = 8
ES = E // N_CORES      # 12500 entity columns per core
ECH = 500              # main-matmul moving chunk (<=512 fp32 psum, divides ES)
NEC = ES // ECH        # 25 chunks
DCH = D // 128         # 8 d-chunks

F32 = mybir.dt.float32
F32R = mybir.dt.float32r    # fp32 data, PE rounds (~tf32)
BF16 = mybir.dt.bfloat16
AF = mybir.AluOpType
ACTF = mybir.ActivationFunctionType
BF16NP = ml_dtypes.bfloat16


def build_nc(has_b2: bool, GC: int):
    """GC = token columns per batch-group (4 groups of 4 batches each)."""
    TP = 4 * GC            # padded active-token count
    TT = TP // 128         # token tiles in the main loop

    nc = bacc.Bacc("TRN2", target_bir_lowering=False, debug=False,
                   enable_asserts=False, num_devices=N_CORES)

    # ---- DRAM I/O (per-core) ----
    d_mask = nc.dram_tensor("maskb", [128, 4 * GC], BF16, kind="ExternalInput").ap()
    d_embg = nc.dram_tensor("emb_g", [128, 4 * D], BF16, kind="ExternalInput").ap()
    d_gamma = nc.dram_tensor("gamma_r", [128, DCH], F32, kind="ExternalInput").ap()
    d_bg = nc.dram_tensor("bg", [128, 2 * DCH], F32, kind="ExternalInput").ap()
    d_w1 = nc.dram_tensor("w1", [128, DCH * R], F32, kind="ExternalInput").ap()
    d_w2 = nc.dram_tensor("w2s", [R, ES], BF16, kind="ExternalInput").ap()
    d_b2 = nc.dram_tensor("b2s", [1, ES], F32, kind="ExternalInput").ap()
    d_onesc = nc.dram_tensor("onesc", [128, 1], F32, kind="ExternalInput").ap()
    d_onesr = nc.dram_tensor("onesr", [1, 128], F32, kind="ExternalInput").ap()
    d_out = nc.dram_tensor("out", [TP, ES], BF16, kind="ExternalOutput").ap()

    def tchunks():
        return [slice(t0, min(t0 + 512, TP)) for t0 in range(0, TP, 512)]

    with tile.TileContext(nc) as tc:
        with (
            tc.tile_pool(name="persist", bufs=1) as pp,
            tc.tile_pool(name="pre", bufs=1) as pre,
        ):
            w2r_sb = pp.tile([R, ES], BF16)
            hT_sb = pp.tile([R, TP], BF16)

            # input DMAs fan out over three rings: w2 on the ACT ring,
            # pooling-critical loads on the sync ring, fold weights on SWDGE.
            nc.scalar.dma_start(w2r_sb[:], d_w2[:])

            mask_sb = pre.tile([128, 4, GC], BF16)
            nc.sync.dma_start(mask_sb[:], d_mask[:])
            embg_sb = pre.tile([128, 4, D], BF16)
            nc.sync.dma_start(embg_sb[:], d_embg[:])
            onesc_sb = pre.tile([128, 1], F32)
            nc.sync.dma_start(onesc_sb[:], d_onesc[:])
            onesr_sb = pre.tile([1, 128], F32)
            nc.sync.dma_start(onesr_sb[:], d_onesr[:])
            w1_sb = pre.tile([128, DCH, R], F32)
            nc.gpsimd.dma_start(w1_sb[:], d_w1.rearrange("p (c r) -> p c r", r=R))
            gamma_sb = pre.tile([128, DCH], F32)
            nc.gpsimd.dma_start(gamma_sb[:], d_gamma[:])
            bg_sb = pre.tile([128, DCH, 2], F32)
            nc.gpsimd.dma_start(bg_sb[:], d_bg.rearrange("p (c two) -> p c two", two=2))

            # PE warm-up: ~4us of dummy matmuls while input DMAs land, so
            # the tensor engine exits its low p-state before pooling.
            warm_sb = pre.tile([128, 512], BF16)
            nc.vector.memset(warm_sb[:], 0.0)
            with tc.tile_pool(name="warmps", bufs=1, space="PSUM") as wps:
                warm_ps = wps.tile([128, 512], F32)
                for _ in range(8):
                    nc.tensor.matmul(out=warm_ps[:], lhsT=warm_sb[:, 0:128],
                                     rhs=warm_sb[:], start=True, stop=True,
                                     skip_group_check=True)

            onescr_sb = pre.tile([128, 1], F32R)
            nc.vector.tensor_copy(onescr_sb[:], onesc_sb[:])
            onesrr_sb = pre.tile([1, 128], F32R)
            nc.vector.tensor_copy(onesrr_sb[:], onesr_sb[:])

            # ---- pooling: pooledT[d, t] = sum_x emb[b(t), x, d] * m[t, x]/L
            # bf16 matmuls; stats matmuls run one d-chunk behind.
            pooledT_sb = pre.tile([128, DCH, TP], F32R)
            mu_sb = pre.tile([1, TP], F32R)
            e2_sb = pre.tile([1, TP], F32R)
            sq_tiles = {}

            def emit_stats(nc, sps_s1, sps_s2, dc):
                for sl in tchunks():
                    nc.tensor.matmul(out=sps_s1[:, sl], lhsT=onescr_sb[:],
                                     rhs=pooledT_sb[:, dc, sl],
                                     start=(dc == 0), stop=(dc == DCH - 1),
                                     skip_group_check=True)
                    nc.tensor.matmul(out=sps_s2[:, sl], lhsT=onescr_sb[:],
                                     rhs=sq_tiles[dc][:, sl],
                                     start=(dc == 0), stop=(dc == DCH - 1),
                                     skip_group_check=True)

            with tc.tile_pool(name="poolps", bufs=2, space="PSUM") as pps, \
                 tc.tile_pool(name="statps", bufs=1, space="PSUM") as sps, \
                 tc.tile_pool(name="sqp", bufs=3) as sqp:
                s1_ps = sps.tile([1, TP], F32)
                s2_ps = sps.tile([1, TP], F32)
                for dc in range(DCH):
                    pt_ps = pps.tile([128, TP], F32, tag="pt")
                    for g in range(4):
                        nc.tensor.matmul(
                            out=pt_ps[:, g * GC:(g + 1) * GC],
                            lhsT=embg_sb[:, g, dc * 128:(dc + 1) * 128],
                            rhs=mask_sb[:, g, :],
                            start=True, stop=True,
                        )
                    nc.vector.tensor_copy(pooledT_sb[:, dc, :], pt_ps[:])
                    sq_tiles[dc] = sqp.tile([128, TP], F32R, tag="sq", name=f"sqt{dc}")
                    nc.scalar.square(sq_tiles[dc][:], pooledT_sb[:, dc, :])
                    if dc >= 1:
                        emit_stats(nc, s1_ps, s2_ps, dc - 1)
                emit_stats(nc, s1_ps, s2_ps, DCH - 1)
                nc.vector.tensor_scalar(mu_sb[:], s1_ps[:], 1.0 / D, None,
                                        op0=AF.mult)
                nc.vector.tensor_scalar(e2_sb[:], s2_ps[:], 1.0 / D, None,
                                        op0=AF.mult)

            with tc.tile_pool(name="foldps", bufs=1, space="PSUM") as fps:
                # ---- classifier folds (PE work independent of stats) ----
                # [c | u] = [beta | gamma] @ w1   (fp32, N=2)
                cu_ps = fps.tile([128, 2], F32)
                for dc in range(DCH):
                    nc.tensor.matmul(out=cu_ps[:], lhsT=w1_sb[:, dc, :],
                                     rhs=bg_sb[:, dc, :],
                                     start=(dc == 0), stop=(dc == DCH - 1),
                                     skip_group_check=True)
                cu_sb = pre.tile([128, 2], F32)
                nc.vector.tensor_copy(cu_sb[:], cu_ps[:])
                # w1g = gamma (.) w1, rounded
                w1g_sb = pre.tile([128, DCH, R], F32R)
                for dc in range(DCH):
                    nc.vector.tensor_scalar(w1g_sb[:, dc, :], w1_sb[:, dc, :],
                                            gamma_sb[:, dc:dc + 1], None,
                                            op0=AF.mult)
                # yT = w1g.T @ pooledT
                yT_ps = fps.tile([128, TP], F32)
                for sl in tchunks():
                    for dc in range(DCH):
                        nc.tensor.matmul(out=yT_ps[:, sl],
                                         lhsT=w1g_sb[:, dc, :],
                                         rhs=pooledT_sb[:, dc, sl],
                                         start=(dc == 0), stop=(dc == DCH - 1),
                                         skip_group_check=True)

                with tc.tile_pool(name="bcps", bufs=1, space="PSUM") as bps:
                    # broadcast mu, E[x^2] across partitions via ones-matmul
                    mub_ps = bps.tile([128, TP], F32)
                    e2b_ps = bps.tile([128, TP], F32)
                    for sl in tchunks():
                        nc.tensor.matmul(out=mub_ps[:, sl], lhsT=onesrr_sb[:],
                                         rhs=mu_sb[:, sl], start=True, stop=True)
                        nc.tensor.matmul(out=e2b_ps[:, sl], lhsT=onesrr_sb[:],
                                         rhs=e2_sb[:, sl], start=True, stop=True)
                    musq_sb = pre.tile([128, TP], F32)
                    nc.scalar.square(musq_sb[:], mub_ps[:])
                    vare_sb = pre.tile([128, TP], F32)
                    # var + eps = (e2b + eps) - musq
                    nc.vector.scalar_tensor_tensor(vare_sb[:], in0=e2b_ps[:],
                                                   scalar=LN_EPS, in1=musq_sb[:],
                                                   op0=AF.add, op1=AF.subtract)
                    # rs = 1/sqrt(var+eps)   (var+eps > 0)
                    rs_sb = pre.tile([128, TP], F32)
                    nc.scalar.activation(rs_sb[:], vare_sb[:],
                                         ACTF.Abs_reciprocal_sqrt)
                    # nmurs = -(mu * rs)
                    nmurs_sb = pre.tile([128, TP], F32)
                    nc.vector.scalar_tensor_tensor(nmurs_sb[:], in0=mub_ps[:],
                                                   scalar=-1.0, in1=rs_sb[:],
                                                   op0=AF.mult, op1=AF.mult)

                # ---- hT = rs*yT + nmurs*u + c  (rounded to bf16) ----
                t1_sb = pre.tile([128, TP], F32)
                t2_sb = pre.tile([128, TP], F32)
                for sl in tchunks():
                    nc.vector.tensor_tensor(t1_sb[:, sl], yT_ps[:, sl],
                                            rs_sb[:, sl], op=AF.mult)
                    nc.vector.scalar_tensor_tensor(t2_sb[:, sl],
                                                   in0=nmurs_sb[:, sl],
                                                   scalar=cu_sb[:, 1:2],
                                                   in1=t1_sb[:, sl],
                                                   op0=AF.mult, op1=AF.add)
                    nc.vector.tensor_scalar(hT_sb[:, sl], t2_sb[:, sl],
                                            cu_sb[:, 0:1], None, op0=AF.add)

            # ---- main: out[t, e] = hT.T @ w2 (+ b2), bf16 out ----
            # full ES-wide rows staged in SBUF; two ~1.6MB DMAs per token
            # tile (second half overlaps the next tile's compute)
            HALF = 12 * ECH       # 6000
            with tc.tile_pool(name="mainps", bufs=8, space="PSUM") as mps2, \
                 tc.tile_pool(name="outp", bufs=2) as op, \
                 tc.tile_pool(name="b2p", bufs=2) as b2p, \
                 tc.tile_pool(name="b2ps", bufs=2, space="PSUM") as b2pp:
                bb_sb = None
                if has_b2:
                    b2c = b2p.tile([1, ES], F32)
                    nc.sync.dma_start(b2c[:], d_b2[:])
                    b2cr = b2p.tile([1, ES], F32R)
                    nc.vector.tensor_copy(b2cr[:], b2c[:])
                    bb_sb = b2p.tile([128, ES], F32)
                    for ec in range(NEC):
                        esl = slice(ec * ECH, (ec + 1) * ECH)
                        bb_ps = b2pp.tile([128, ECH], F32, tag="bbp")
                        nc.tensor.matmul(out=bb_ps[:], lhsT=onesrr_sb[:],
                                         rhs=b2cr[:, esl], start=True, stop=True)
                        nc.vector.tensor_copy(bb_sb[:, esl], bb_ps[:])
                for tt in range(TT):
                    o_sb = op.tile([128, ES], BF16, tag="o")
                    trow = slice(tt * 128, (tt + 1) * 128)
                    for ec in range(NEC):
                        esl = slice(ec * ECH, (ec + 1) * ECH)
                        mm_ps = mps2.tile([128, ECH], F32, tag="mm")
                        nc.tensor.matmul(out=mm_ps[:],
                                         lhsT=hT_sb[:, tt * 128:(tt + 1) * 128],
                                         rhs=w2r_sb[:, esl],
                                         start=True, stop=True)
                        even = (tt * NEC + ec) % 2 == 0
                        if has_b2:
                            eng = nc.vector.tensor_tensor if even else \
                                nc.gpsimd.tensor_tensor
                            eng(o_sb[:, esl], mm_ps[:], bb_sb[:, esl], op=AF.add)
                        else:
                            # alternate evacuation engine: DVE / ACT
                            if even:
                                nc.vector.tensor_copy(o_sb[:, esl], mm_ps[:])
                            else:
                                nc.scalar.copy(o_sb[:, esl], mm_ps[:])
                        if ec == HALF // ECH - 1:
                            dma_eng = nc.sync if tt % 2 == 0 else nc.scalar
                            dma_eng.dma_start(d_out[trow, 0:HALF],
                                              o_sb[:, 0:HALF])
                    dma_eng = nc.scalar if tt % 2 == 0 else nc.sync
                    dma_eng.dma_start(d_out[trow, HALF:ES], o_sb[:, HALF:ES])

    nc.finalize()
    return nc


_NC_CACHE = {}


def _get_nc(has_b2: bool, GC: int):
    key = (has_b2, GC)
    if key not in _NC_CACHE:
        _NC_CACHE[key] = build_nc(has_b2, GC)
    return _NC_CACHE[key]


def prep_core_inputs(inputs):
    """Host-side compaction/layout prep.

    Returns (shared_map, per_core_w2, per_core_b2, meta) where meta carries
    (has_b2, GC, dev_rows [n_act], tok_idx [n_act], const_row [E]).
    """
    ids = np.asarray(inputs["input_ids"]).astype(np.int64)[:, :L]      # [16, 32]
    pos = np.asarray(inputs["entity_position_ids"])                    # [B, M, L]
    am = np.asarray(inputs["entity_attention_mask"])                   # [B, M]
    embed = np.asarray(inputs["embed"], dtype=np.float32)
    gamma = np.asarray(inputs["ln_gamma"], dtype=np.float32)
    beta = np.asarray(inputs["ln_beta"], dtype=np.float32)
    w1 = np.asarray(inputs["w1"], dtype=np.float32)
    w2 = np.asarray(inputs["w2"], dtype=np.float32)
    b2 = np.asarray(inputs["b2"], dtype=np.float32)

    mrow = (pos != -1)                                                 # [B, M, L]
    active = (am != 0) & mrow.any(-1)                                  # [B, M]

    # group g holds batches 4g..4g+3 on partitions 32k..32k+32 (k = b - 4g)
    tok_lists = []
    for g in range(4):
        toks = []
        for k in range(4):
            b = 4 * g + k
            for j in np.nonzero(active[b])[0]:
                toks.append((k, b, int(j)))
        tok_lists.append(toks)
    n_max = max(1, max(len(t) for t in tok_lists))
    GC = 128 * ((n_max + 127) // 128)
    TP = 4 * GC

    mask_blk = np.zeros((128, 4, GC), np.float32)
    dev_rows = []                     # device row of each active token
    tok_idx = []                      # flat token index b*M + j
    for g in range(4):
        for c, (k, b, j) in enumerate(tok_lists[g]):
            mask_blk[32 * k:32 * k + 32, g, c] = mrow[b, j] / np.float32(L)
            dev_rows.append(g * GC + c)
            tok_idx.append(b * M + j)

    # emb_g[32k + x, g, :] = embed[ids[4g + k, x]]
    emb_idx = ids.reshape(4, 4, L).transpose(1, 2, 0).reshape(128, 4)
    emb_g = np.ascontiguousarray(
        embed[emb_idx].reshape(128, 4 * D)).astype(BF16NP)
    gamma_r = np.ascontiguousarray(gamma.reshape(DCH, 128).T)          # [128, 8]
    beta_r = np.ascontiguousarray(beta.reshape(DCH, 128).T)
    bg = np.ascontiguousarray(
        np.stack([beta_r, gamma_r], axis=-1).reshape(128, 2 * DCH))

    # w1 in [128, DCH*R] layout: line p = [w1[c*128+p, r] for c, r]
    w1_dev = np.ascontiguousarray(
        w1.reshape(DCH, 128, R).transpose(1, 0, 2).reshape(128, DCH * R))
    shared = {
        "maskb": np.ascontiguousarray(mask_blk.reshape(128, 4 * GC)).astype(BF16NP),
        "emb_g": emb_g,
        "gamma_r": gamma_r,
        "bg": bg,
        "w1": w1_dev,
        "onesc": np.ones((128, 1), np.float32),
        "onesr": np.ones((1, 128), np.float32),
    }
    w2s = [np.ascontiguousarray(w2[:, c * ES:(c + 1) * ES]).astype(BF16NP)
           for c in range(N_CORES)]
    b2s = [np.ascontiguousarray(b2[c * ES:(c + 1) * ES].reshape(1, ES))
           for c in range(N_CORES)]
    has_b2 = bool(np.any(b2 != 0.0))
    # masked mentions all produce LayerNorm(0) = beta -> (beta@w1)@w2 + b2
    const_row = (beta @ w1) @ w2 + b2                                  # [E] f32
    meta = {
        "has_b2": has_b2,
        "GC": GC,
        "dev_rows": np.asarray(dev_rows, np.int64),
        "tok_idx": np.asarray(tok_idx, np.int64),
        "const_row": const_row.astype(np.float32),
        "active": active,
    }
    return shared, w2s, b2s, meta


def _bf16_to_f32(a):
    return (a.view(np.uint16).astype(np.uint32) << 16).view(np.float32)


def kernel(**inputs) -> np.ndarray:
    shared, w2s, b2s, meta = prep_core_inputs(inputs)
    nc = _get_nc(meta["has_b2"], meta["GC"])
    in_maps = [dict(shared, w2s=w2s[c], b2s=b2s[c]) for c in range(N_CORES)]
    res = run_bass_kernel_spmd(nc, in_maps, list(range(N_CORES)))

    full = np.zeros((B * M, E), np.float32)
    dev_rows, tok_idx = meta["dev_rows"], meta["tok_idx"]
    if len(tok_idx):
        buf = np.empty((len(tok_idx), E), np.float32)
        for c in range(N_CORES):
            blk = np.asarray(res.results[c]["out"])[dev_rows]   # bf16 [n_act, ES]
            buf[:, c * ES:(c + 1) * ES] = _bf16_to_f32(
                np.ascontiguousarray(blk))
        full[tok_idx] = buf
    cr = meta["const_row"]
    if np.any(cr != 0.0):
        inactive = np.nonzero(~meta["active"].reshape(-1))[0]
        full[inactive] = cr
    return np.ascontiguousarray(full.reshape(B, M, E))
